# revision 11
# baseline (speedup 1.0000x reference)
"""EuclideanCodebook (VQ + EMA update) Trainium2 kernel.

Strategy (8 NeuronCores, data-parallel over tokens):
  - Each core gets 4096 tokens (x shard), full codebook replicated.
  - Scores s[t,k] = 2*x.e_k - |e_k|^2 computed as three fp16 matmul passes
    (fp16(x)*fp16(2e) + fp16(x)*fp16(d2e) + fp16(dx)*fp16(2e), where d* are
    the fp16 rounding residuals) accumulated in fp32 PSUM on top of an
    ACT-prefilled -|e|^2 bias.  Combined error ~2e-6 sigma, enough to
    reproduce the reference fp32 argmax exactly.
  - argmax via DVE MAX8 + FIND_INDEX8 straight from PSUM.
  - quantize via indirect-DMA gather of exact fp32 embed rows.
  - counts/embed_sum via fp16 one-hot (iota == ind) + PE matmuls accumulated
    in PSUM across all tiles; AllReduce'd across the 8 cores; EMA update
    computed on-device (replicated), outputs taken from core 0.
"""

import os
import sys

import numpy as np

if "/opt/trn_rl_repo" not in sys.path:
    sys.path.insert(0, "/opt/trn_rl_repo")

B, T, D, K = 8, 4096, 128, 1024
NCORES = 8
N_TOK = B * T
TOK_PER_CORE = N_TOK // NCORES
P = 128
NT = TOK_PER_CORE // P  # tiles of 128 tokens per core
DECAY = 0.99
EPSILON = 1e-5

_CACHE = {}


def _maybe_enable_trace():
    """Install the axon NTFF profile hook if tracing was requested via
    BASS_TRACE=1 (used by the local test harness; harmless otherwise)."""
    if os.environ.get("BASS_TRACE") != "1":
        return
    try:
        import types
        import antenv

        if "antenv.axon_hooks" not in sys.modules:
            mod = types.ModuleType("antenv.axon_hooks")
            _hook = [None]
            mod.set_axon_ntff_profile_hook = lambda h: _hook.__setitem__(0, h)
            mod.get_axon_ntff_profile_hook = lambda: _hook[0]
            sys.modules["antenv.axon_hooks"] = mod
            antenv.axon_hooks = mod
        if "/root/.axon_site" not in sys.path:
            sys.path.insert(0, "/root/.axon_site")
        from trn_agent_boot.trn_boot import _ntff_profile_via_ctypes

        hook = _ntff_profile_via_ctypes("/opt/axon/libaxon_pjrt.so")
        if hook is not None:
            sys.modules["antenv.axon_hooks"].set_axon_ntff_profile_hook(hook)
        import concourse.bass_utils as bu

        bu.upload_artifacts = lambda tmpdir: "file://" + str(tmpdir)
    except Exception:
        pass


def _build_nc():
    import concourse.bacc as bacc
    import concourse.bass as bass
    import concourse.mybir as mybir
    import concourse.tile as tile
    from concourse import bass_isa
    from concourse.masks import make_identity

    f32 = mybir.dt.float32
    f16 = mybir.dt.float16
    i32 = mybir.dt.int32
    u32 = mybir.dt.uint32
    Alu = mybir.AluOpType
    Act = mybir.ActivationFunctionType

    nc = bacc.Bacc("TRN2", target_bir_lowering=False, debug=False, num_devices=NCORES)

    # ---- I/O ----
    xh_t = nc.dram_tensor("xh_t", [D, TOK_PER_CORE], f16, kind="ExternalInput")
    dx_t = nc.dram_tensor("dx_t", [D, TOK_PER_CORE], f16, kind="ExternalInput")
    xn16 = nc.dram_tensor("xn16", [TOK_PER_CORE, D], f16, kind="ExternalInput")
    eh_t = nc.dram_tensor("eh_t", [D, K], f16, kind="ExternalInput")
    de_t = nc.dram_tensor("de_t", [D, K], f16, kind="ExternalInput")
    ne2f = nc.dram_tensor("ne2f", [P, K], f32, kind="ExternalInput")
    embed_in = nc.dram_tensor("embed_in", [K, D], f32, kind="ExternalInput")
    cs_in = nc.dram_tensor("cs_in", [K], f32, kind="ExternalInput")
    ea_in = nc.dram_tensor("ea_in", [K, D], f32, kind="ExternalInput")

    q_out = nc.dram_tensor("q_out", [TOK_PER_CORE, D], f32, kind="ExternalOutput")
    ind_out = nc.dram_tensor("ind_out", [TOK_PER_CORE], f32, kind="ExternalOutput")
    ncs_out = nc.dram_tensor("ncs_out", [K], f32, kind="ExternalOutput")
    nea_out = nc.dram_tensor("nea_out", [K, D], f32, kind="ExternalOutput")
    ne_out = nc.dram_tensor("ne_out", [K, D], f32, kind="ExternalOutput")

    # collective bounce buffers (must be internal DRAM)
    cc_in = nc.dram_tensor("cc_in", [P + 1, K], f32)
    cc_out = nc.dram_tensor("cc_out", [P + 1, K], f32)

    NB = K // P  # 8 codebook blocks

    with tile.TileContext(nc) as tc:
        with tc.tile_pool(name="const", bufs=1) as cpool, \
             tc.tile_pool(name="espsum", bufs=1, space="PSUM") as es_pool:
            # ---- constants ----
            eh_sb = cpool.tile([D, K], f16)
            de_sb = cpool.tile([D, K], f16)
            ne2_sb = cpool.tile([P, K], f32)
            nc.sync.dma_start(eh_sb[:], eh_t[:])
            nc.sync.dma_start(de_sb[:], de_t[:])
            nc.sync.dma_start(ne2_sb[:], ne2f[:])

            iota_i = cpool.tile([P, K], i32)
            nc.gpsimd.iota(iota_i[:], pattern=[[1, K]], base=0, channel_multiplier=0)
            iota16 = cpool.tile([P, K], f16)
            nc.vector.tensor_copy(iota16[:], iota_i[:])

            ones16 = cpool.tile([P, 1], f16)
            nc.vector.memset(ones16[:], 1.0)
            oh_acc = cpool.tile([P, K], f16)
            nc.vector.memset(oh_acc[:], 0.0)
            ident = cpool.tile([P, P], f32)
            make_identity(nc, ident[:])

            # persistent PSUM accumulator for embed_sum^T [d, k]
            ps_es = es_pool.tile([P, K], f32, space="PSUM")

            z16 = cpool.tile([1, P], f16)
            nc.vector.memset(z16[:], 0.0)

            # prefetch all x data for this core into SBUF up front
            xh_all = cpool.tile([D, TOK_PER_CORE], f16)
            dx_all = cpool.tile([D, TOK_PER_CORE], f16)
            xn_all = cpool.tile([P, NT * D], f16)
            nc.sync.dma_start(xh_all[:], xh_t[:])
            nc.sync.dma_start(dx_all[:], dx_t[:])
            nc.sync.dma_start(
                xn_all[:].rearrange("p (i d) -> p i d", i=NT),
                xn16[:].rearrange("(i p) d -> p i d", p=P))
            ind_all = cpool.tile([P, NT], f32)

            # ---- main loop over 32 token tiles ----
            with tc.tile_pool(name="work", bufs=4) as wpool, \
                 tc.tile_pool(name="spsum", bufs=3, space="PSUM") as sp_pool:
                # Prime the has_written bits of the score PSUM buffers with a
                # start=True zero matmul so the per-tile start=False matmuls
                # accumulate onto the ACT-prefilled bias instead of replacing it.
                for _ in range(3):
                    psd = sp_pool.tile([P, K], f32, space="PSUM", tag="ps")
                    for j in range(2):
                        sl = slice(j * 512, (j + 1) * 512)
                        nc.tensor.matmul(psd[:, sl], lhsT=z16[:], rhs=eh_sb[0:1, sl],
                                         start=True, stop=True)
                for i in range(NT):
                    t0, t1 = i * P, (i + 1) * P
                    xh_i = xh_all[:, t0:t1]
                    dx_i = dx_all[:, t0:t1]
                    xn_i = xn_all[:, i * D:(i + 1) * D]

                    ps = sp_pool.tile([P, K], f32, space="PSUM", tag="ps")
                    nc.scalar.activation(ps[:], ne2_sb[:], Act.Copy)
                    for j in range(2):
                        sl = slice(j * 512, (j + 1) * 512)
                        nc.tensor.matmul(ps[:, sl], lhsT=xh_i, rhs=eh_sb[:, sl],
                                         start=False, stop=False)
                        nc.tensor.matmul(ps[:, sl], lhsT=xh_i, rhs=de_sb[:, sl],
                                         start=False, stop=False)
                        nc.tensor.matmul(ps[:, sl], lhsT=dx_i, rhs=eh_sb[:, sl],
                                         start=False, stop=True)

                    m8 = wpool.tile([P, 8], f32, tag="m8")
                    i8 = wpool.tile([P, 8], u32, tag="i8")
                    nc.vector.max(out=m8[:], in_=ps[:])
                    nc.vector.max_index(out=i8[:], in_max=m8[:], in_values=ps[:])

                    indf = ind_all[:, i:i + 1]
                    nc.vector.tensor_copy(indf, i8[:, 0:1])

                    oh = wpool.tile([P, K], f16, tag="oh")
                    nc.vector.tensor_scalar(
                        out=oh[:], in0=iota16[:], scalar1=indf, scalar2=0.0,
                        op0=Alu.subtract, op1=Alu.is_equal)
                    nc.gpsimd.tensor_add(oh_acc[:], oh_acc[:], oh[:])

                    for j in range(2):
                        sl = slice(j * 512, (j + 1) * 512)
                        nc.tensor.matmul(ps_es[:, sl], lhsT=xn_i, rhs=oh[:, sl],
                                         start=(i == 0), stop=(i == NT - 1))

                    qrow = wpool.tile([P, D], f32, tag="qrow")
                    nc.gpsimd.indirect_dma_start(
                        out=qrow[:], out_offset=None, in_=embed_in[:],
                        in_offset=bass.IndirectOffsetOnAxis(ap=i8[:, :1], axis=0))
                    nc.scalar.dma_start(q_out[t0:t1, :], qrow[:])

            # ---- tail: counts, all-reduce, EMA ----
            with tc.tile_pool(name="tail", bufs=1) as tpool, \
                 tc.tile_pool(name="tpsum", bufs=1, space="PSUM") as tp_pool:
                # indices out: transpose [p, i] -> [i, p] = flat token order
                ps_i = tp_pool.tile([NT, P], f32, space="PSUM", tag="pi")
                nc.tensor.transpose(ps_i[:], ind_all[:], ident[:])
                idxb = tpool.tile([NT, P], f32)
                nc.vector.tensor_copy(idxb[:], ps_i[:])
                nc.sync.dma_start(ind_out[:].rearrange("(i p) -> i p", p=P), idxb[:])

                ps_c = tp_pool.tile([1, K], f32, space="PSUM", tag="pc")
                for j in range(2):
                    sl = slice(j * 512, (j + 1) * 512)
                    nc.tensor.matmul(ps_c[:, sl], lhsT=ones16[:], rhs=oh_acc[:, sl],
                                     start=True, stop=True)
                esT_sb = tpool.tile([P, K], f32)
                cnt_sb = tpool.tile([1, K], f32)
                nc.vector.tensor_copy(esT_sb[:], ps_es[:])
                nc.vector.tensor_copy(cnt_sb[:], ps_c[:])
                nc.sync.dma_start(cc_in[0:P, :], esT_sb[:])
                nc.sync.dma_start(cc_in[P:P + 1, :], cnt_sb[:])
                nc.gpsimd.collective_compute(
                    "AllReduce", mybir.AluOpType.add,
                    replica_groups=[list(range(NCORES))],
                    ins=[cc_in[:].opt()], outs=[cc_out[:].opt()])
                esT_all = tpool.tile([P, K], f32)
                cntb = tpool.tile([NB, P], f32)
                nc.sync.dma_start(esT_all[:], cc_out[0:P, :])
                nc.sync.dma_start(cntb[:], cc_out[P, :].rearrange("(b p) -> b p", p=P))

                csb = tpool.tile([NB, P], f32)
                nc.sync.dma_start(csb[:], cs_in[:].rearrange("(b p) -> b p", p=P))
                ea_sb = tpool.tile([P, NB * D], f32)
                nc.sync.dma_start(
                    ea_sb[:].rearrange("p (b d) -> p b d", b=NB),
                    ea_in[:].rearrange("(b p) d -> p b d", p=P))

                # transpose embed_sum^T back to [k, d] blocks, and counts/cs to [p, b]
                es_nat = tpool.tile([P, NB * D], f32)
                for b in range(NB):
                    pst = tp_pool.tile([P, P], f32, space="PSUM", tag="ptr")
                    nc.tensor.transpose(pst[:], esT_all[:, b * P:(b + 1) * P], ident[:])
                    nc.vector.tensor_copy(es_nat[:, b * D:(b + 1) * D], pst[:])
                cnt8 = tpool.tile([P, NB], f32)
                cs8 = tpool.tile([P, NB], f32)
                pst = tp_pool.tile([P, NB], f32, space="PSUM", tag="ptr2")
                nc.tensor.transpose(pst[:], cntb[:], ident[:NB, :NB])
                nc.vector.tensor_copy(cnt8[:], pst[:])
                pst2 = tp_pool.tile([P, NB], f32, space="PSUM", tag="ptr2")
                nc.tensor.transpose(pst2[:], csb[:], ident[:NB, :NB])
                nc.vector.tensor_copy(cs8[:], pst2[:])

                # ncs = decay*cs + (1-decay)*counts
                ncs8 = tpool.tile([P, NB], f32)
                tmp8 = tpool.tile([P, NB], f32)
                nc.vector.tensor_scalar(out=ncs8[:], in0=cs8[:], scalar1=DECAY,
                                        scalar2=None, op0=Alu.mult)
                nc.vector.tensor_scalar(out=tmp8[:], in0=cnt8[:], scalar1=1.0 - DECAY,
                                        scalar2=None, op0=Alu.mult)
                nc.vector.tensor_add(ncs8[:], ncs8[:], tmp8[:])

                # total = sum(ncs) broadcast to all partitions
                part = tpool.tile([P, 1], f32)
                nc.vector.reduce_sum(part[:], ncs8[:], axis=mybir.AxisListType.X)
                tot = tpool.tile([P, 1], f32)
                nc.gpsimd.partition_all_reduce(tot[:], part[:], channels=P,
                                               reduce_op=bass_isa.ReduceOp.add)

                def newton_recip(dst, src, scratch):
                    nc.vector.reciprocal(dst[:], src[:])
                    nc.vector.tensor_mul(scratch[:], src[:], dst[:])
                    nc.vector.tensor_scalar(out=scratch[:], in0=scratch[:], scalar1=-1.0,
                                            scalar2=2.0, op0=Alu.mult, op1=Alu.add)
                    nc.vector.tensor_mul(dst[:], dst[:], scratch[:])

                denom = tpool.tile([P, 1], f32)
                nc.vector.tensor_scalar(out=denom[:], in0=tot[:], scalar1=K * EPSILON,
                                        scalar2=None, op0=Alu.add)
                rden = tpool.tile([P, 1], f32)
                scr1 = tpool.tile([P, 1], f32)
                newton_recip(rden, denom, scr1)
                fmul = tpool.tile([P, 1], f32)
                nc.vector.tensor_mul(fmul[:], tot[:], rden[:])

                sm8 = tpool.tile([P, NB], f32)
                nc.vector.tensor_scalar(out=sm8[:], in0=ncs8[:], scalar1=EPSILON,
                                        scalar2=None, op0=Alu.add)
                nc.vector.tensor_scalar(out=sm8[:], in0=sm8[:], scalar1=fmul[:],
                                        scalar2=None, op0=Alu.mult)
                inv8 = tpool.tile([P, NB], f32)
                scr8 = tpool.tile([P, NB], f32)
                newton_recip(inv8, sm8, scr8)

                # nea = decay*ea + (1-decay)*embed_sum ; ne = nea * inv_smoothed
                nea = tpool.tile([P, NB * D], f32)
                nc.vector.tensor_scalar(out=nea[:], in0=es_nat[:], scalar1=1.0 - DECAY,
                                        scalar2=None, op0=Alu.mult)
                tmp = tpool.tile([P, NB * D], f32)
                nc.vector.tensor_scalar(out=tmp[:], in0=ea_sb[:], scalar1=DECAY,
                                        scalar2=None, op0=Alu.mult)
                nc.vector.tensor_add(nea[:], nea[:], tmp[:])
                ne = tpool.tile([P, NB * D], f32)
                for b in range(NB):
                    nc.vector.tensor_scalar(
                        out=ne[:, b * D:(b + 1) * D], in0=nea[:, b * D:(b + 1) * D],
                        scalar1=inv8[:, b:b + 1], scalar2=None, op0=Alu.mult)

                # outputs
                nc.sync.dma_start(
                    nea_out[:].rearrange("(b p) d -> p b d", p=P),
                    nea[:].rearrange("p (b d) -> p b d", b=NB))
                nc.sync.dma_start(
                    ne_out[:].rearrange("(b p) d -> p b d", p=P),
                    ne[:].rearrange("p (b d) -> p b d", b=NB))
                # ncs: transpose [p, b] -> [b, p] then contiguous store
                pst3 = tp_pool.tile([NB, P], f32, space="PSUM", tag="ptr3")
                nc.tensor.transpose(pst3[:], ncs8[:], ident[:])
                ncsb = tpool.tile([NB, P], f32)
                nc.vector.tensor_copy(ncsb[:], pst3[:])
                nc.sync.dma_start(ncs_out[:].rearrange("(b p) -> b p", p=P), ncsb[:])

    nc.compile()
    return nc


def _prep_inputs(x, embed, cluster_size, embed_avg):
    x = np.ascontiguousarray(np.asarray(x, dtype=np.float32).reshape(-1, D))
    e = np.asarray(embed, dtype=np.float32)
    cs = np.asarray(cluster_size, dtype=np.float32).reshape(-1)
    ea = np.ascontiguousarray(np.asarray(embed_avg, dtype=np.float32))

    xh16 = x.astype(np.float16)
    dx16 = (x - xh16.astype(np.float32)).astype(np.float16)
    twoe = 2.0 * e
    eh16 = twoe.astype(np.float16)
    de16 = (twoe - eh16.astype(np.float32)).astype(np.float16)
    eh_t = np.ascontiguousarray(eh16.T)
    de_t = np.ascontiguousarray(de16.T)
    e2 = (e.astype(np.float64) ** 2).sum(1).astype(np.float32)
    ne2f = np.ascontiguousarray(np.repeat(-e2[None, :], P, 0))

    in_maps = []
    for c in range(NCORES):
        s0, s1 = c * TOK_PER_CORE, (c + 1) * TOK_PER_CORE
        in_maps.append({
            "xh_t": np.ascontiguousarray(xh16[s0:s1].T),
            "dx_t": np.ascontiguousarray(dx16[s0:s1].T),
            "xn16": np.ascontiguousarray(xh16[s0:s1]),
            "eh_t": eh_t,
            "de_t": de_t,
            "ne2f": ne2f,
            "embed_in": e,
            "cs_in": cs,
            "ea_in": ea,
        })
    return in_maps


def kernel(x, embed, cluster_size, embed_avg):
    _maybe_enable_trace()
    if "nc" not in _CACHE:
        _CACHE["nc"] = _build_nc()
    nc = _CACHE["nc"]

    from concourse.bass_utils import run_bass_kernel_spmd

    in_maps = _prep_inputs(x, embed, cluster_size, embed_avg)
    res = run_bass_kernel_spmd(nc, in_maps, core_ids=list(range(NCORES)))
    if res.exec_time_ns is not None:
        kernel.last_exec_time_ns = res.exec_time_ns
        print(f"HW exec time: {res.exec_time_ns} ns")
    kernel.last_results = res

    shape = np.asarray(x).shape
    r = res.results
    quantize = np.concatenate([r[c]["q_out"] for c in range(NCORES)], 0).reshape(shape)
    embed_ind = np.concatenate(
        [r[c]["ind_out"] for c in range(NCORES)], 0).astype(np.int32).reshape(shape[:-1])
    new_cluster_size = r[0]["ncs_out"]
    new_embed_avg = r[0]["nea_out"]
    new_embed = r[0]["ne_out"]
    return quantize, embed_ind, new_cluster_size, new_embed_avg, new_embed


# revision 13
# speedup vs baseline: 1.0928x; 1.0928x over previous
"""EuclideanCodebook (VQ + EMA update) Trainium2 kernel.

Strategy (8 NeuronCores, data-parallel over tokens):
  - Each core gets 4096 tokens (x shard), full codebook replicated.
  - Scores s[t,k] = 2*x.e_k - |e_k|^2 computed as three fp16 matmul passes
    (fp16(x)*fp16(2e) + fp16(x)*fp16(d2e) + fp16(dx)*fp16(2e), where d* are
    the fp16 rounding residuals) accumulated in fp32 PSUM on top of an
    ACT-prefilled -|e|^2 bias.  Combined error ~2e-6 sigma, enough to
    reproduce the reference fp32 argmax exactly.
  - argmax via DVE MAX8 + FIND_INDEX8 straight from PSUM.
  - quantize via indirect-DMA gather of exact fp32 embed rows.
  - counts/embed_sum via fp16 one-hot (iota == ind) + PE matmuls accumulated
    in PSUM across all tiles; AllReduce'd across the 8 cores; EMA update
    computed on-device (replicated), outputs taken from core 0.
"""

import os
import sys

import numpy as np

if "/opt/trn_rl_repo" not in sys.path:
    sys.path.insert(0, "/opt/trn_rl_repo")

B, T, D, K = 8, 4096, 128, 1024
NCORES = 8
N_TOK = B * T
TOK_PER_CORE = N_TOK // NCORES
P = 128
NT = TOK_PER_CORE // P  # tiles of 128 tokens per core
DECAY = 0.99
EPSILON = 1e-5

_CACHE = {}


def _maybe_enable_trace():
    """Install the axon NTFF profile hook if tracing was requested via
    BASS_TRACE=1 (used by the local test harness; harmless otherwise)."""
    if os.environ.get("BASS_TRACE") != "1":
        return
    try:
        import types
        import antenv

        if "antenv.axon_hooks" not in sys.modules:
            mod = types.ModuleType("antenv.axon_hooks")
            _hook = [None]
            mod.set_axon_ntff_profile_hook = lambda h: _hook.__setitem__(0, h)
            mod.get_axon_ntff_profile_hook = lambda: _hook[0]
            sys.modules["antenv.axon_hooks"] = mod
            antenv.axon_hooks = mod
        if "/root/.axon_site" not in sys.path:
            sys.path.insert(0, "/root/.axon_site")
        from trn_agent_boot.trn_boot import _ntff_profile_via_ctypes

        hook = _ntff_profile_via_ctypes("/opt/axon/libaxon_pjrt.so")
        if hook is not None:
            sys.modules["antenv.axon_hooks"].set_axon_ntff_profile_hook(hook)
        import concourse.bass_utils as bu

        bu.upload_artifacts = lambda tmpdir: "file://" + str(tmpdir)
    except Exception:
        pass


def _build_nc():
    import concourse.bacc as bacc
    import concourse.bass as bass
    import concourse.mybir as mybir
    import concourse.tile as tile
    from concourse import bass_isa
    from concourse.masks import make_identity

    f32 = mybir.dt.float32
    f16 = mybir.dt.float16
    i32 = mybir.dt.int32
    u32 = mybir.dt.uint32
    Alu = mybir.AluOpType
    Act = mybir.ActivationFunctionType

    nc = bacc.Bacc("TRN2", target_bir_lowering=False, debug=False, num_devices=NCORES)

    # ---- I/O ----
    xh_t = nc.dram_tensor("xh_t", [D, TOK_PER_CORE], f16, kind="ExternalInput")
    dx_t = nc.dram_tensor("dx_t", [D, TOK_PER_CORE], f16, kind="ExternalInput")
    xn16 = nc.dram_tensor("xn16", [TOK_PER_CORE, D], f16, kind="ExternalInput")
    eh_t = nc.dram_tensor("eh_t", [D, K], f16, kind="ExternalInput")
    de_t = nc.dram_tensor("de_t", [D, K], f16, kind="ExternalInput")
    ne2f = nc.dram_tensor("ne2f", [P, K], f32, kind="ExternalInput")
    embed_in = nc.dram_tensor("embed_in", [K, D], f32, kind="ExternalInput")
    cs_in = nc.dram_tensor("cs_in", [K], f32, kind="ExternalInput")
    ea_in = nc.dram_tensor("ea_in", [K, D], f32, kind="ExternalInput")

    q_out = nc.dram_tensor("q_out", [TOK_PER_CORE, D], f32, kind="ExternalOutput")
    ind_out = nc.dram_tensor("ind_out", [TOK_PER_CORE], f32, kind="ExternalOutput")
    ncs_out = nc.dram_tensor("ncs_out", [K], f32, kind="ExternalOutput")
    nea_out = nc.dram_tensor("nea_out", [K, D], f32, kind="ExternalOutput")
    ne_out = nc.dram_tensor("ne_out", [K, D], f32, kind="ExternalOutput")

    # collective bounce buffers (must be internal DRAM)
    cc_in = nc.dram_tensor("cc_in", [P + 1, K], f32)
    cc_out = nc.dram_tensor("cc_out", [P + 1, K], f32)

    NB = K // P  # 8 codebook blocks

    with tile.TileContext(nc) as tc:
        with tc.tile_pool(name="const", bufs=1) as cpool, \
             tc.tile_pool(name="espsum", bufs=1, space="PSUM") as es_pool:
            # ---- constants ----
            eh_sb = cpool.tile([D, K], f16)
            de_sb = cpool.tile([D, K], f16)
            ne2_sb = cpool.tile([P, K], f32)
            nc.sync.dma_start(eh_sb[:], eh_t[:])
            nc.sync.dma_start(de_sb[:], de_t[:])
            nc.sync.dma_start(ne2_sb[:], ne2f[:])

            iota_i = cpool.tile([P, K], i32)
            nc.gpsimd.iota(iota_i[:], pattern=[[1, K]], base=0, channel_multiplier=0)
            iota16 = cpool.tile([P, K], f16)
            nc.vector.tensor_copy(iota16[:], iota_i[:])

            ones16 = cpool.tile([P, 1], f16)
            nc.vector.memset(ones16[:], 1.0)
            oh_acc = cpool.tile([P, K], f16)
            nc.vector.memset(oh_acc[:], 0.0)
            ident = cpool.tile([P, P], f32)
            make_identity(nc, ident[:])

            # persistent PSUM accumulator for embed_sum^T [d, k]
            ps_es = es_pool.tile([P, K], f32, space="PSUM")

            z16 = cpool.tile([1, P], f16)
            nc.vector.memset(z16[:], 0.0)

            # prefetch all x data for this core into SBUF up front
            xh_all = cpool.tile([D, TOK_PER_CORE], f16)
            dx_all = cpool.tile([D, TOK_PER_CORE], f16)
            xn_all = cpool.tile([P, NT * D], f16)
            nc.sync.dma_start(xh_all[:], xh_t[:])
            nc.sync.dma_start(dx_all[:], dx_t[:])
            nc.sync.dma_start(
                xn_all[:].rearrange("p (i d) -> p i d", i=NT),
                xn16[:].rearrange("(i p) d -> p i d", p=P))
            ind_all = cpool.tile([P, NT], f32)

            # ---- main loop over 32 token tiles ----
            with tc.tile_pool(name="work", bufs=4) as wpool, \
                 tc.tile_pool(name="spsum", bufs=3, space="PSUM") as sp_pool:
                # Prime the has_written bits of the score PSUM buffers with a
                # start=True zero matmul so the per-tile start=False matmuls
                # accumulate onto the ACT-prefilled bias instead of replacing it.
                for _ in range(3):
                    psd = sp_pool.tile([P, K], f32, space="PSUM", tag="ps")
                    for j in range(2):
                        sl = slice(j * 512, (j + 1) * 512)
                        nc.tensor.matmul(psd[:, sl], lhsT=z16[:], rhs=eh_sb[0:1, sl],
                                         start=True, stop=True)
                for i in range(NT):
                    t0, t1 = i * P, (i + 1) * P
                    xh_i = xh_all[:, t0:t1]
                    dx_i = dx_all[:, t0:t1]
                    xn_i = xn_all[:, i * D:(i + 1) * D]

                    ps = sp_pool.tile([P, K], f32, space="PSUM", tag="ps")
                    nc.scalar.activation(ps[:], ne2_sb[:], Act.Copy)
                    for j in range(2):
                        sl = slice(j * 512, (j + 1) * 512)
                        nc.tensor.matmul(ps[:, sl], lhsT=xh_i, rhs=eh_sb[:, sl],
                                         start=False, stop=False)
                        nc.tensor.matmul(ps[:, sl], lhsT=xh_i, rhs=de_sb[:, sl],
                                         start=False, stop=False)
                        nc.tensor.matmul(ps[:, sl], lhsT=dx_i, rhs=eh_sb[:, sl],
                                         start=False, stop=True)

                    m8 = wpool.tile([P, 8], f32, tag="m8")
                    i8 = wpool.tile([P, 8], u32, tag="i8")
                    nc.vector.max(out=m8[:], in_=ps[:])
                    nc.vector.max_index(out=i8[:], in_max=m8[:], in_values=ps[:])

                    indf = ind_all[:, i:i + 1]
                    nc.vector.tensor_copy(indf, i8[:, 0:1])

                    oh = wpool.tile([P, K], f16, tag="oh")
                    nc.vector.tensor_scalar(
                        out=oh[:], in0=iota16[:], scalar1=indf, scalar2=0.0,
                        op0=Alu.subtract, op1=Alu.is_equal)
                    nc.vector.tensor_add(oh_acc[:], oh_acc[:], oh[:])

                    for j in range(2):
                        sl = slice(j * 512, (j + 1) * 512)
                        nc.tensor.matmul(ps_es[:, sl], lhsT=xn_i, rhs=oh[:, sl],
                                         start=(i == 0), stop=(i == NT - 1))

                    qrow = wpool.tile([P, D], f32, tag="qrow")
                    nc.gpsimd.indirect_dma_start(
                        out=qrow[:], out_offset=None, in_=embed_in[:],
                        in_offset=bass.IndirectOffsetOnAxis(ap=i8[:, :1], axis=0))
                    nc.scalar.dma_start(q_out[t0:t1, :], qrow[:])

            # ---- tail: counts, all-reduce, EMA ----
            with tc.tile_pool(name="tail", bufs=1) as tpool, \
                 tc.tile_pool(name="tpsum", bufs=1, space="PSUM") as tp_pool:
                # indices out: transpose [p, i] -> [i, p] = flat token order
                ps_i = tp_pool.tile([NT, P], f32, space="PSUM", tag="ptr")
                nc.tensor.transpose(ps_i[:], ind_all[:], ident[:])
                idxb = tpool.tile([NT, P], f32)
                nc.vector.tensor_copy(idxb[:], ps_i[:])
                nc.sync.dma_start(ind_out[:].rearrange("(i p) -> i p", p=P), idxb[:])

                ps_c = tp_pool.tile([1, K], f32, space="PSUM", tag="pc")
                for j in range(2):
                    sl = slice(j * 512, (j + 1) * 512)
                    nc.tensor.matmul(ps_c[:, sl], lhsT=ones16[:], rhs=oh_acc[:, sl],
                                     start=True, stop=True)
                esT_sb = tpool.tile([P, K], f32)
                cnt_sb = tpool.tile([1, K], f32)
                nc.vector.tensor_copy(esT_sb[:], ps_es[:])
                nc.vector.tensor_copy(cnt_sb[:], ps_c[:])
                nc.sync.dma_start(cc_in[0:P, :], esT_sb[:])
                nc.sync.dma_start(cc_in[P:P + 1, :], cnt_sb[:])
                nc.gpsimd.collective_compute(
                    "AllReduce", mybir.AluOpType.add,
                    replica_groups=[list(range(NCORES))],
                    ins=[cc_in[:].opt()], outs=[cc_out[:].opt()])
                esT_all = tpool.tile([P, K], f32)
                cntb = tpool.tile([NB, P], f32)
                nc.sync.dma_start(esT_all[:], cc_out[0:P, :])
                nc.sync.dma_start(cntb[:], cc_out[P, :].rearrange("(b p) -> b p", p=P))

                csb = tpool.tile([NB, P], f32)
                nc.sync.dma_start(csb[:], cs_in[:].rearrange("(b p) -> b p", p=P))
                ea_sb = tpool.tile([P, NB * D], f32)
                nc.sync.dma_start(
                    ea_sb[:].rearrange("p (b d) -> p b d", b=NB),
                    ea_in[:].rearrange("(b p) d -> p b d", p=P))

                # transpose embed_sum^T back to [k, d] blocks, and counts/cs to [p, b]
                es_nat = tpool.tile([P, NB * D], f32)
                for b in range(NB):
                    pst = tp_pool.tile([P, P], f32, space="PSUM", tag="ptr")
                    nc.tensor.transpose(pst[:], esT_all[:, b * P:(b + 1) * P], ident[:])
                    nc.vector.tensor_copy(es_nat[:, b * D:(b + 1) * D], pst[:])
                cnt8 = tpool.tile([P, NB], f32)
                cs8 = tpool.tile([P, NB], f32)
                pst = tp_pool.tile([P, NB], f32, space="PSUM", tag="ptr2")
                nc.tensor.transpose(pst[:], cntb[:], ident[:NB, :NB])
                nc.vector.tensor_copy(cnt8[:], pst[:])
                pst2 = tp_pool.tile([P, NB], f32, space="PSUM", tag="ptr2")
                nc.tensor.transpose(pst2[:], csb[:], ident[:NB, :NB])
                nc.vector.tensor_copy(cs8[:], pst2[:])

                # ncs = decay*cs + (1-decay)*counts
                ncs8 = tpool.tile([P, NB], f32)
                tmp8 = tpool.tile([P, NB], f32)
                nc.vector.tensor_scalar(out=ncs8[:], in0=cs8[:], scalar1=DECAY,
                                        scalar2=None, op0=Alu.mult)
                nc.vector.tensor_scalar(out=tmp8[:], in0=cnt8[:], scalar1=1.0 - DECAY,
                                        scalar2=None, op0=Alu.mult)
                nc.vector.tensor_add(ncs8[:], ncs8[:], tmp8[:])

                # total = sum(ncs) broadcast to all partitions
                part = tpool.tile([P, 1], f32)
                nc.vector.reduce_sum(part[:], ncs8[:], axis=mybir.AxisListType.X)
                ones32 = tpool.tile([P, 1], f32)
                nc.vector.memset(ones32[:], 1.0)
                ps_t1 = tp_pool.tile([1, 1], f32, space="PSUM", tag="ptr2")
                nc.tensor.matmul(ps_t1[:], lhsT=part[:], rhs=ones32[:],
                                 start=True, stop=True)
                tot1 = tpool.tile([1, 1], f32)
                nc.vector.tensor_copy(tot1[:], ps_t1[:])
                ps_t2 = tp_pool.tile([P, 1], f32, space="PSUM", tag="ptr2")
                onerow = tpool.tile([1, P], f32)
                nc.vector.memset(onerow[:], 1.0)
                nc.tensor.matmul(ps_t2[:], lhsT=onerow[:], rhs=tot1[:],
                                 start=True, stop=True)
                tot = tpool.tile([P, 1], f32)
                nc.vector.tensor_copy(tot[:], ps_t2[:])

                def newton_recip(dst, src, scratch):
                    nc.vector.reciprocal(dst[:], src[:])
                    nc.vector.tensor_mul(scratch[:], src[:], dst[:])
                    nc.vector.tensor_scalar(out=scratch[:], in0=scratch[:], scalar1=-1.0,
                                            scalar2=2.0, op0=Alu.mult, op1=Alu.add)
                    nc.vector.tensor_mul(dst[:], dst[:], scratch[:])

                denom = tpool.tile([P, 1], f32)
                nc.vector.tensor_scalar(out=denom[:], in0=tot[:], scalar1=K * EPSILON,
                                        scalar2=None, op0=Alu.add)
                rden = tpool.tile([P, 1], f32)
                scr1 = tpool.tile([P, 1], f32)
                newton_recip(rden, denom, scr1)
                fmul = tpool.tile([P, 1], f32)
                nc.vector.tensor_mul(fmul[:], tot[:], rden[:])

                sm8 = tpool.tile([P, NB], f32)
                nc.vector.tensor_scalar(out=sm8[:], in0=ncs8[:], scalar1=EPSILON,
                                        scalar2=None, op0=Alu.add)
                nc.vector.tensor_scalar(out=sm8[:], in0=sm8[:], scalar1=fmul[:],
                                        scalar2=None, op0=Alu.mult)
                inv8 = tpool.tile([P, NB], f32)
                scr8 = tpool.tile([P, NB], f32)
                newton_recip(inv8, sm8, scr8)

                # nea = decay*ea + (1-decay)*embed_sum ; ne = nea * inv_smoothed
                nea = tpool.tile([P, NB * D], f32)
                nc.vector.tensor_scalar(out=nea[:], in0=es_nat[:], scalar1=1.0 - DECAY,
                                        scalar2=None, op0=Alu.mult)
                tmp = tpool.tile([P, NB * D], f32)
                nc.vector.tensor_scalar(out=tmp[:], in0=ea_sb[:], scalar1=DECAY,
                                        scalar2=None, op0=Alu.mult)
                nc.vector.tensor_add(nea[:], nea[:], tmp[:])
                ne = tpool.tile([P, NB * D], f32)
                for b in range(NB):
                    nc.vector.tensor_scalar(
                        out=ne[:, b * D:(b + 1) * D], in0=nea[:, b * D:(b + 1) * D],
                        scalar1=inv8[:, b:b + 1], scalar2=None, op0=Alu.mult)

                # outputs
                nc.sync.dma_start(
                    nea_out[:].rearrange("(b p) d -> p b d", p=P),
                    nea[:].rearrange("p (b d) -> p b d", b=NB))
                nc.sync.dma_start(
                    ne_out[:].rearrange("(b p) d -> p b d", p=P),
                    ne[:].rearrange("p (b d) -> p b d", b=NB))
                # ncs: transpose [p, b] -> [b, p] then contiguous store
                pst3 = tp_pool.tile([NB, P], f32, space="PSUM", tag="ptr3")
                nc.tensor.transpose(pst3[:], ncs8[:], ident[:])
                ncsb = tpool.tile([NB, P], f32)
                nc.vector.tensor_copy(ncsb[:], pst3[:])
                nc.sync.dma_start(ncs_out[:].rearrange("(b p) -> b p", p=P), ncsb[:])

    nc.compile()
    return nc


def _prep_inputs(x, embed, cluster_size, embed_avg):
    x = np.ascontiguousarray(np.asarray(x, dtype=np.float32).reshape(-1, D))
    e = np.asarray(embed, dtype=np.float32)
    cs = np.asarray(cluster_size, dtype=np.float32).reshape(-1)
    ea = np.ascontiguousarray(np.asarray(embed_avg, dtype=np.float32))

    xh16 = x.astype(np.float16)
    dx16 = (x - xh16.astype(np.float32)).astype(np.float16)
    twoe = 2.0 * e
    eh16 = twoe.astype(np.float16)
    de16 = (twoe - eh16.astype(np.float32)).astype(np.float16)
    eh_t = np.ascontiguousarray(eh16.T)
    de_t = np.ascontiguousarray(de16.T)
    e2 = (e.astype(np.float64) ** 2).sum(1).astype(np.float32)
    ne2f = np.ascontiguousarray(np.repeat(-e2[None, :], P, 0))

    in_maps = []
    for c in range(NCORES):
        s0, s1 = c * TOK_PER_CORE, (c + 1) * TOK_PER_CORE
        in_maps.append({
            "xh_t": np.ascontiguousarray(xh16[s0:s1].T),
            "dx_t": np.ascontiguousarray(dx16[s0:s1].T),
            "xn16": np.ascontiguousarray(xh16[s0:s1]),
            "eh_t": eh_t,
            "de_t": de_t,
            "ne2f": ne2f,
            "embed_in": e,
            "cs_in": cs,
            "ea_in": ea,
        })
    return in_maps


def kernel(x, embed, cluster_size, embed_avg):
    _maybe_enable_trace()
    if "nc" not in _CACHE:
        _CACHE["nc"] = _build_nc()
    nc = _CACHE["nc"]

    from concourse.bass_utils import run_bass_kernel_spmd

    in_maps = _prep_inputs(x, embed, cluster_size, embed_avg)
    res = run_bass_kernel_spmd(nc, in_maps, core_ids=list(range(NCORES)))
    if res.exec_time_ns is not None:
        kernel.last_exec_time_ns = res.exec_time_ns
        print(f"HW exec time: {res.exec_time_ns} ns")
    kernel.last_results = res

    shape = np.asarray(x).shape
    r = res.results
    quantize = np.concatenate([r[c]["q_out"] for c in range(NCORES)], 0).reshape(shape)
    embed_ind = np.concatenate(
        [r[c]["ind_out"] for c in range(NCORES)], 0).astype(np.int32).reshape(shape[:-1])
    new_cluster_size = r[0]["ncs_out"]
    new_embed_avg = r[0]["nea_out"]
    new_embed = r[0]["ne_out"]
    return quantize, embed_ind, new_cluster_size, new_embed_avg, new_embed


# revision 14
# speedup vs baseline: 1.1402x; 1.0434x over previous
"""EuclideanCodebook (VQ + EMA update) Trainium2 kernel.

Strategy (8 NeuronCores, data-parallel over tokens):
  - Each core gets 4096 tokens (x shard), full codebook replicated.
  - Scores s[t,k] = 2*x.e_k - |e_k|^2 computed as three fp16 matmul passes
    (fp16(x)*fp16(2e) + fp16(x)*fp16(d2e) + fp16(dx)*fp16(2e), where d* are
    the fp16 rounding residuals) accumulated in fp32 PSUM on top of an
    ACT-prefilled -|e|^2 bias.  Combined error ~2e-6 sigma, enough to
    reproduce the reference fp32 argmax exactly.
  - argmax via DVE MAX8 + FIND_INDEX8 straight from PSUM.
  - quantize via indirect-DMA gather of exact fp32 embed rows.
  - counts/embed_sum via fp16 one-hot (iota == ind) + PE matmuls accumulated
    in PSUM across all tiles; AllReduce'd across the 8 cores; EMA update
    computed on-device (replicated), outputs taken from core 0.
"""

import os
import sys

import numpy as np

if "/opt/trn_rl_repo" not in sys.path:
    sys.path.insert(0, "/opt/trn_rl_repo")

B, T, D, K = 8, 4096, 128, 1024
NCORES = 8
N_TOK = B * T
TOK_PER_CORE = N_TOK // NCORES
P = 128
NT = TOK_PER_CORE // P  # tiles of 128 tokens per core
DECAY = 0.99
EPSILON = 1e-5

_CACHE = {}


def _maybe_enable_trace():
    """Install the axon NTFF profile hook if tracing was requested via
    BASS_TRACE=1 (used by the local test harness; harmless otherwise)."""
    if os.environ.get("BASS_TRACE") != "1":
        return
    try:
        import types
        import antenv

        if "antenv.axon_hooks" not in sys.modules:
            mod = types.ModuleType("antenv.axon_hooks")
            _hook = [None]
            mod.set_axon_ntff_profile_hook = lambda h: _hook.__setitem__(0, h)
            mod.get_axon_ntff_profile_hook = lambda: _hook[0]
            sys.modules["antenv.axon_hooks"] = mod
            antenv.axon_hooks = mod
        if "/root/.axon_site" not in sys.path:
            sys.path.insert(0, "/root/.axon_site")
        from trn_agent_boot.trn_boot import _ntff_profile_via_ctypes

        hook = _ntff_profile_via_ctypes("/opt/axon/libaxon_pjrt.so")
        if hook is not None:
            sys.modules["antenv.axon_hooks"].set_axon_ntff_profile_hook(hook)
        import concourse.bass_utils as bu

        bu.upload_artifacts = lambda tmpdir: "file://" + str(tmpdir)
    except Exception:
        pass


def _build_nc():
    import concourse.bacc as bacc
    import concourse.bass as bass
    import concourse.mybir as mybir
    import concourse.tile as tile
    from concourse import bass_isa
    from concourse.masks import make_identity

    f32 = mybir.dt.float32
    f16 = mybir.dt.float16
    i32 = mybir.dt.int32
    u32 = mybir.dt.uint32
    Alu = mybir.AluOpType
    Act = mybir.ActivationFunctionType

    nc = bacc.Bacc("TRN2", target_bir_lowering=False, debug=False, num_devices=NCORES)

    # ---- I/O ----
    xh_t = nc.dram_tensor("xh_t", [D, TOK_PER_CORE], f16, kind="ExternalInput")
    dx_t = nc.dram_tensor("dx_t", [D, TOK_PER_CORE], f16, kind="ExternalInput")
    xn16 = nc.dram_tensor("xn16", [TOK_PER_CORE, D], f16, kind="ExternalInput")
    eh_t = nc.dram_tensor("eh_t", [D, K], f16, kind="ExternalInput")
    de_t = nc.dram_tensor("de_t", [D, K], f16, kind="ExternalInput")
    ne2f = nc.dram_tensor("ne2f", [P, K], f32, kind="ExternalInput")
    embed_in = nc.dram_tensor("embed_in", [K, D], f32, kind="ExternalInput")
    cs_in = nc.dram_tensor("cs_in", [K], f32, kind="ExternalInput")
    ea_in = nc.dram_tensor("ea_in", [K, D], f32, kind="ExternalInput")

    q_out = nc.dram_tensor("q_out", [TOK_PER_CORE, D], f32, kind="ExternalOutput")
    ind_out = nc.dram_tensor("ind_out", [TOK_PER_CORE], f32, kind="ExternalOutput")
    ncs_out = nc.dram_tensor("ncs_out", [K], f32, kind="ExternalOutput")
    nea_out = nc.dram_tensor("nea_out", [K, D], f32, kind="ExternalOutput")
    ne_out = nc.dram_tensor("ne_out", [K, D], f32, kind="ExternalOutput")

    # collective bounce buffers (must be internal DRAM)
    cc_in = nc.dram_tensor("cc_in", [P + 1, K], f32)
    cc_out = nc.dram_tensor("cc_out", [P + 1, K], f32)

    NB = K // P  # 8 codebook blocks

    with tile.TileContext(nc) as tc:
        with tc.tile_pool(name="const", bufs=1) as cpool, \
             tc.tile_pool(name="espsum", bufs=1, space="PSUM") as es_pool:
            # ---- constants ----
            eh_sb = cpool.tile([D, K], f16)
            de_sb = cpool.tile([D, K], f16)
            ne2_sb = cpool.tile([P, K], f32)
            nc.sync.dma_start(eh_sb[:], eh_t[:])
            nc.sync.dma_start(de_sb[:], de_t[:])
            nc.sync.dma_start(ne2_sb[:], ne2f[:])

            iota_i = cpool.tile([P, K], i32)
            nc.gpsimd.iota(iota_i[:], pattern=[[1, K]], base=0, channel_multiplier=0)
            iota16 = cpool.tile([P, K], f16)
            nc.vector.tensor_copy(iota16[:], iota_i[:])

            ones16 = cpool.tile([P, 1], f16)
            nc.vector.memset(ones16[:], 1.0)
            oh_acc = cpool.tile([P, K], f16)
            nc.vector.memset(oh_acc[:], 0.0)
            ident = cpool.tile([P, P], f32)
            make_identity(nc, ident[:])

            # persistent PSUM accumulator for embed_sum^T [d, k]
            ps_es = es_pool.tile([P, K], f32, space="PSUM")

            z16 = cpool.tile([1, P], f16)
            nc.vector.memset(z16[:], 0.0)

            # prefetch all x data for this core into SBUF up front
            xh_all = cpool.tile([D, TOK_PER_CORE], f16)
            dx_all = cpool.tile([D, TOK_PER_CORE], f16)
            xn_all = cpool.tile([P, NT * D], f16)
            nc.sync.dma_start(xh_all[:], xh_t[:])
            nc.sync.dma_start(dx_all[:], dx_t[:])
            nc.sync.dma_start(
                xn_all[:].rearrange("p (i d) -> p i d", i=NT),
                xn16[:].rearrange("(i p) d -> p i d", p=P))
            ind_all = cpool.tile([P, NT], f32)

            # ---- main loop over 32 token tiles ----
            with tc.tile_pool(name="work", bufs=4) as wpool, \
                 tc.tile_pool(name="spsum", bufs=3, space="PSUM") as sp_pool:
                # Prime the has_written bits of the score PSUM buffers with a
                # start=True zero matmul so the per-tile start=False matmuls
                # accumulate onto the ACT-prefilled bias instead of replacing it.
                for _ in range(3):
                    psd = sp_pool.tile([P, K], f32, space="PSUM", tag="ps")
                    for j in range(2):
                        sl = slice(j * 512, (j + 1) * 512)
                        nc.tensor.matmul(psd[:, sl], lhsT=z16[:], rhs=eh_sb[0:1, sl],
                                         start=True, stop=True)
                def emit_esum(i, oh):
                    xn_i = xn_all[:, i * D:(i + 1) * D]
                    for j in range(2):
                        sl = slice(j * 512, (j + 1) * 512)
                        nc.tensor.matmul(ps_es[:, sl], lhsT=xn_i, rhs=oh[:, sl],
                                         start=(i == 0), stop=(i == NT - 1))

                prev = None  # (i, oh)
                for i in range(NT):
                    t0, t1 = i * P, (i + 1) * P
                    xh_i = xh_all[:, t0:t1]
                    dx_i = dx_all[:, t0:t1]

                    ps = sp_pool.tile([P, K], f32, space="PSUM", tag="ps")
                    nc.scalar.activation(ps[:], ne2_sb[:], Act.Copy)
                    for j in range(2):
                        sl = slice(j * 512, (j + 1) * 512)
                        nc.tensor.matmul(ps[:, sl], lhsT=xh_i, rhs=eh_sb[:, sl],
                                         start=False, stop=False)
                        nc.tensor.matmul(ps[:, sl], lhsT=xh_i, rhs=de_sb[:, sl],
                                         start=False, stop=False)
                        nc.tensor.matmul(ps[:, sl], lhsT=dx_i, rhs=eh_sb[:, sl],
                                         start=False, stop=True)
                    # previous tile's embed_sum matmuls go behind this tile's
                    # score matmuls so the PE FIFO never waits on oh_i
                    if prev is not None:
                        emit_esum(*prev)

                    m8 = wpool.tile([P, 8], f32, tag="m8")
                    i8 = wpool.tile([P, 8], u32, tag="i8")
                    nc.vector.max(out=m8[:], in_=ps[:])
                    nc.vector.max_index(out=i8[:], in_max=m8[:], in_values=ps[:])

                    indf = ind_all[:, i:i + 1]
                    nc.vector.tensor_copy(indf, i8[:, 0:1])

                    oh = wpool.tile([P, K], f16, tag="oh")
                    nc.vector.tensor_scalar(
                        out=oh[:], in0=iota16[:], scalar1=indf, scalar2=0.0,
                        op0=Alu.subtract, op1=Alu.is_equal)
                    nc.vector.tensor_add(oh_acc[:], oh_acc[:], oh[:])
                    prev = (i, oh)

                    qrow = wpool.tile([P, D], f32, tag="qrow")
                    nc.gpsimd.indirect_dma_start(
                        out=qrow[:], out_offset=None, in_=embed_in[:],
                        in_offset=bass.IndirectOffsetOnAxis(ap=i8[:, :1], axis=0))
                    nc.sync.dma_start(q_out[t0:t1, :], qrow[:])
                emit_esum(*prev)

            # ---- tail: counts, all-reduce, EMA ----
            with tc.tile_pool(name="tail", bufs=1) as tpool, \
                 tc.tile_pool(name="tpsum", bufs=1, space="PSUM") as tp_pool:
                # indices out: transpose [p, i] -> [i, p] = flat token order
                ps_i = tp_pool.tile([NT, P], f32, space="PSUM", tag="ptr")
                nc.tensor.transpose(ps_i[:], ind_all[:], ident[:])
                idxb = tpool.tile([NT, P], f32)
                nc.vector.tensor_copy(idxb[:], ps_i[:])
                nc.sync.dma_start(ind_out[:].rearrange("(i p) -> i p", p=P), idxb[:])

                ps_c = tp_pool.tile([1, K], f32, space="PSUM", tag="pc")
                for j in range(2):
                    sl = slice(j * 512, (j + 1) * 512)
                    nc.tensor.matmul(ps_c[:, sl], lhsT=ones16[:], rhs=oh_acc[:, sl],
                                     start=True, stop=True)
                esT_sb = tpool.tile([P, K], f32)
                cnt_sb = tpool.tile([1, K], f32)
                nc.vector.tensor_copy(esT_sb[:], ps_es[:])
                nc.vector.tensor_copy(cnt_sb[:], ps_c[:])
                nc.sync.dma_start(cc_in[0:P, :], esT_sb[:])
                nc.sync.dma_start(cc_in[P:P + 1, :], cnt_sb[:])
                nc.gpsimd.collective_compute(
                    "AllReduce", mybir.AluOpType.add,
                    replica_groups=[list(range(NCORES))],
                    ins=[cc_in[:].opt()], outs=[cc_out[:].opt()])
                esT_all = tpool.tile([P, K], f32)
                cntb = tpool.tile([NB, P], f32)
                nc.sync.dma_start(esT_all[:], cc_out[0:P, :])
                nc.sync.dma_start(cntb[:], cc_out[P, :].rearrange("(b p) -> b p", p=P))

                csb = tpool.tile([NB, P], f32)
                nc.sync.dma_start(csb[:], cs_in[:].rearrange("(b p) -> b p", p=P))
                ea_sb = tpool.tile([P, NB * D], f32)
                nc.sync.dma_start(
                    ea_sb[:].rearrange("p (b d) -> p b d", b=NB),
                    ea_in[:].rearrange("(b p) d -> p b d", p=P))

                # transpose embed_sum^T back to [k, d] blocks, and counts/cs to [p, b]
                es_nat = tpool.tile([P, NB * D], f32)
                for b in range(NB):
                    pst = tp_pool.tile([P, P], f32, space="PSUM", tag="ptr")
                    nc.tensor.transpose(pst[:], esT_all[:, b * P:(b + 1) * P], ident[:])
                    nc.vector.tensor_copy(es_nat[:, b * D:(b + 1) * D], pst[:])
                cnt8 = tpool.tile([P, NB], f32)
                cs8 = tpool.tile([P, NB], f32)
                pst = tp_pool.tile([P, NB], f32, space="PSUM", tag="ptr2")
                nc.tensor.transpose(pst[:], cntb[:], ident[:NB, :NB])
                nc.vector.tensor_copy(cnt8[:], pst[:])
                pst2 = tp_pool.tile([P, NB], f32, space="PSUM", tag="ptr2")
                nc.tensor.transpose(pst2[:], csb[:], ident[:NB, :NB])
                nc.vector.tensor_copy(cs8[:], pst2[:])

                # ncs = decay*cs + (1-decay)*counts
                ncs8 = tpool.tile([P, NB], f32)
                tmp8 = tpool.tile([P, NB], f32)
                nc.vector.tensor_scalar(out=ncs8[:], in0=cs8[:], scalar1=DECAY,
                                        scalar2=None, op0=Alu.mult)
                nc.vector.tensor_scalar(out=tmp8[:], in0=cnt8[:], scalar1=1.0 - DECAY,
                                        scalar2=None, op0=Alu.mult)
                nc.vector.tensor_add(ncs8[:], ncs8[:], tmp8[:])

                # total = sum(ncs) broadcast to all partitions
                part = tpool.tile([P, 1], f32)
                nc.vector.reduce_sum(part[:], ncs8[:], axis=mybir.AxisListType.X)
                ones32 = tpool.tile([P, 1], f32)
                nc.vector.memset(ones32[:], 1.0)
                ps_t1 = tp_pool.tile([1, 1], f32, space="PSUM", tag="ptr2")
                nc.tensor.matmul(ps_t1[:], lhsT=part[:], rhs=ones32[:],
                                 start=True, stop=True)
                tot1 = tpool.tile([1, 1], f32)
                nc.vector.tensor_copy(tot1[:], ps_t1[:])
                ps_t2 = tp_pool.tile([P, 1], f32, space="PSUM", tag="ptr2")
                onerow = tpool.tile([1, P], f32)
                nc.vector.memset(onerow[:], 1.0)
                nc.tensor.matmul(ps_t2[:], lhsT=onerow[:], rhs=tot1[:],
                                 start=True, stop=True)
                tot = tpool.tile([P, 1], f32)
                nc.vector.tensor_copy(tot[:], ps_t2[:])

                def newton_recip(dst, src, scratch):
                    nc.vector.reciprocal(dst[:], src[:])
                    nc.vector.tensor_mul(scratch[:], src[:], dst[:])
                    nc.vector.tensor_scalar(out=scratch[:], in0=scratch[:], scalar1=-1.0,
                                            scalar2=2.0, op0=Alu.mult, op1=Alu.add)
                    nc.vector.tensor_mul(dst[:], dst[:], scratch[:])

                denom = tpool.tile([P, 1], f32)
                nc.vector.tensor_scalar(out=denom[:], in0=tot[:], scalar1=K * EPSILON,
                                        scalar2=None, op0=Alu.add)
                rden = tpool.tile([P, 1], f32)
                scr1 = tpool.tile([P, 1], f32)
                newton_recip(rden, denom, scr1)
                fmul = tpool.tile([P, 1], f32)
                nc.vector.tensor_mul(fmul[:], tot[:], rden[:])

                sm8 = tpool.tile([P, NB], f32)
                nc.vector.tensor_scalar(out=sm8[:], in0=ncs8[:], scalar1=EPSILON,
                                        scalar2=None, op0=Alu.add)
                nc.vector.tensor_scalar(out=sm8[:], in0=sm8[:], scalar1=fmul[:],
                                        scalar2=None, op0=Alu.mult)
                inv8 = tpool.tile([P, NB], f32)
                scr8 = tpool.tile([P, NB], f32)
                newton_recip(inv8, sm8, scr8)

                # nea = decay*ea + (1-decay)*embed_sum ; ne = nea * inv_smoothed
                nea = tpool.tile([P, NB * D], f32)
                nc.vector.tensor_scalar(out=nea[:], in0=es_nat[:], scalar1=1.0 - DECAY,
                                        scalar2=None, op0=Alu.mult)
                tmp = tpool.tile([P, NB * D], f32)
                nc.vector.tensor_scalar(out=tmp[:], in0=ea_sb[:], scalar1=DECAY,
                                        scalar2=None, op0=Alu.mult)
                nc.vector.tensor_add(nea[:], nea[:], tmp[:])
                ne = tpool.tile([P, NB * D], f32)
                for b in range(NB):
                    nc.vector.tensor_scalar(
                        out=ne[:, b * D:(b + 1) * D], in0=nea[:, b * D:(b + 1) * D],
                        scalar1=inv8[:, b:b + 1], scalar2=None, op0=Alu.mult)

                # outputs
                nc.sync.dma_start(
                    nea_out[:].rearrange("(b p) d -> p b d", p=P),
                    nea[:].rearrange("p (b d) -> p b d", b=NB))
                nc.sync.dma_start(
                    ne_out[:].rearrange("(b p) d -> p b d", p=P),
                    ne[:].rearrange("p (b d) -> p b d", b=NB))
                # ncs: transpose [p, b] -> [b, p] then contiguous store
                pst3 = tp_pool.tile([NB, P], f32, space="PSUM", tag="ptr3")
                nc.tensor.transpose(pst3[:], ncs8[:], ident[:])
                ncsb = tpool.tile([NB, P], f32)
                nc.vector.tensor_copy(ncsb[:], pst3[:])
                nc.sync.dma_start(ncs_out[:].rearrange("(b p) -> b p", p=P), ncsb[:])

    nc.compile()
    return nc


def _prep_inputs(x, embed, cluster_size, embed_avg):
    x = np.ascontiguousarray(np.asarray(x, dtype=np.float32).reshape(-1, D))
    e = np.asarray(embed, dtype=np.float32)
    cs = np.asarray(cluster_size, dtype=np.float32).reshape(-1)
    ea = np.ascontiguousarray(np.asarray(embed_avg, dtype=np.float32))

    xh16 = x.astype(np.float16)
    dx16 = (x - xh16.astype(np.float32)).astype(np.float16)
    twoe = 2.0 * e
    eh16 = twoe.astype(np.float16)
    de16 = (twoe - eh16.astype(np.float32)).astype(np.float16)
    eh_t = np.ascontiguousarray(eh16.T)
    de_t = np.ascontiguousarray(de16.T)
    e2 = (e.astype(np.float64) ** 2).sum(1).astype(np.float32)
    ne2f = np.ascontiguousarray(np.repeat(-e2[None, :], P, 0))

    in_maps = []
    for c in range(NCORES):
        s0, s1 = c * TOK_PER_CORE, (c + 1) * TOK_PER_CORE
        in_maps.append({
            "xh_t": np.ascontiguousarray(xh16[s0:s1].T),
            "dx_t": np.ascontiguousarray(dx16[s0:s1].T),
            "xn16": np.ascontiguousarray(xh16[s0:s1]),
            "eh_t": eh_t,
            "de_t": de_t,
            "ne2f": ne2f,
            "embed_in": e,
            "cs_in": cs,
            "ea_in": ea,
        })
    return in_maps


def kernel(x, embed, cluster_size, embed_avg):
    _maybe_enable_trace()
    if "nc" not in _CACHE:
        _CACHE["nc"] = _build_nc()
    nc = _CACHE["nc"]

    from concourse.bass_utils import run_bass_kernel_spmd

    in_maps = _prep_inputs(x, embed, cluster_size, embed_avg)
    res = run_bass_kernel_spmd(nc, in_maps, core_ids=list(range(NCORES)))
    if res.exec_time_ns is not None:
        kernel.last_exec_time_ns = res.exec_time_ns
        print(f"HW exec time: {res.exec_time_ns} ns")
    kernel.last_results = res

    shape = np.asarray(x).shape
    r = res.results
    quantize = np.concatenate([r[c]["q_out"] for c in range(NCORES)], 0).reshape(shape)
    embed_ind = np.concatenate(
        [r[c]["ind_out"] for c in range(NCORES)], 0).astype(np.int32).reshape(shape[:-1])
    new_cluster_size = r[0]["ncs_out"]
    new_embed_avg = r[0]["nea_out"]
    new_embed = r[0]["ne_out"]
    return quantize, embed_ind, new_cluster_size, new_embed_avg, new_embed
